# revision 11
# baseline (speedup 1.0000x reference)
"""Complex self-attention on 8 Trainium2 NeuronCores (Bass/Tile), v2.

Model (reference): complex linear q/k/v projections of (x_re, x_im), attention
scores = (Re(q)·Re(k) + Im(q)·Im(k))/sqrt(D), softmax, attn applied to Re(v)
and Im(v), complex output projection. B=2, N=2048, C=1024, H=16, D=64.

Sharding: heads tensor-parallel across 8 cores (2 heads/core, both batches).

v2 changes over the fp32r baseline (724µs):
  - bf16 everywhere (weights, x, q/k/v, exp-scores, attention outputs, o-proj
    matrices). PE rate is identical to fp32r (1 cycle/row) but DMA bytes halve,
    FWL (fast weight load) activates, and SBUF pressure halves. PSUM
    accumulation stays fp32.
  - Host pre-permutes every DRAM tensor into partition-major layout so each
    DMA is 128 large contiguous descriptors (the baseline's 512B-chunk gathers
    made the first x+weight load a 55µs stall).
  - All projections (both batches) run first; attention follows with the full
    8 PSUM banks: scores for a PAIR of key-tiles accumulate into one
    [128,1024] PSUM tile so a single Exp activation covers 1024 elements
    (halves ACT instruction overhead; scalar was pacing the attention loop).
  - den (softmax denominator) and av PSUM pools are double-buffered so the
    slow DVE reciprocal is off the PE critical path.
  - The AllToAll is split per batch ([8,256,256] bf16, ~1MB/core each):
    A2A(b0) overlaps batch-1 attention; A2A(b1) overlaps the b0 output
    projection. The baseline's single 4MB fp32 A2A exposed a 108µs stall.
"""

import sys

if "/opt/trn_rl_repo" not in sys.path:
    sys.path.insert(0, "/opt/trn_rl_repo")

from contextlib import ExitStack

import ml_dtypes
import numpy as np

import concourse.mybir as mybir
import concourse.tile as tile
from concourse import bacc
from concourse.bass_utils import run_bass_kernel_spmd

B, N, C = 2, 2048, 1024
H, D = 16, 64
T = B * N  # 4096 tokens total
NCORES = 8
HPC = H // NCORES  # 2 heads per core
TF = 512  # projection token-chunk (free dim)
NCHT = T // TF  # 8 chunks total over both batches
KT = 2 * C // 128  # 16 contraction tiles of 128 over [x_re; x_im]
TOKB = 256  # A2A token block (per batch, per core slice)
TSL = 512  # final per-core output token count (256 from each batch)
F32 = mybir.dt.float32
BF16 = mybir.dt.bfloat16
BF = ml_dtypes.bfloat16


def _host_prep(inp):
    """Pre-permute all tensors to partition-major bf16 for contiguous DMA."""
    f32 = np.float32
    x_re = np.asarray(inp["x_re"], f32).reshape(T, C)
    x_im = np.asarray(inp["x_im"], f32).reshape(T, C)
    xT2 = np.concatenate([x_re.T, x_im.T], axis=0)  # [2C, T]
    # [2C, T] -> [chunk, p, kt, t] so each 512-token chunk is one contiguous
    # 16KB block per partition
    xc = np.ascontiguousarray(
        xT2.reshape(KT, 128, NCHT, TF).transpose(2, 1, 0, 3).astype(BF)
    )

    per_core = []
    for c in range(NCORES):
        d = {}
        h0 = c * HPC
        ch = slice(h0 * D, (h0 + HPC) * D)
        for nm in ("q", "k", "v"):
            Wre = np.asarray(inp[f"{nm}_Wre"], f32)[ch]  # [128, C]
            Wim = np.asarray(inp[f"{nm}_Wim"], f32)[ch]
            bre = np.asarray(inp[f"{nm}_bre"], f32)[ch]
            bim = np.asarray(inp[f"{nm}_bim"], f32)[ch]
            Ws, bs = [], []
            for hh in range(HPC):
                hs = slice(hh * D, (hh + 1) * D)
                wr = np.concatenate([Wre[hs].T, -Wim[hs].T], axis=0)  # [2C, 64]
                wi = np.concatenate([Wim[hs].T, Wre[hs].T], axis=0)
                Ws.append(np.concatenate([wr, wi], axis=1))  # [2C, 128]
                bs.append(np.concatenate([bre[hs] - bim[hs], bre[hs] + bim[hs]]))
            if nm == "v":
                wv = np.concatenate(Ws, axis=1)  # [2C, 256]
                d["wv"] = np.ascontiguousarray(
                    wv.reshape(KT, 128, 2 * HPC * D).transpose(1, 0, 2).astype(BF)
                )  # [128, KT, 256]
            else:
                w = np.stack(Ws)  # [HPC, 2C, 128]
                d[f"w{nm}"] = np.ascontiguousarray(
                    w.reshape(HPC, KT, 128, 128).transpose(2, 0, 1, 3).astype(BF)
                )  # [128, HPC, KT, 128]
                d[f"b{nm}"] = np.ascontiguousarray(np.stack(bs, axis=1))  # [128, HPC]
        per_core.append(d)

    # o-projection combined matrices, rows ordered to match the A2A result:
    # rank r, then per rank [h0:out_r(64), h0:out_i(64), h1:out_r(64), h1:out_i(64)]
    oWre = np.asarray(inp["o_Wre"], f32)
    oWim = np.asarray(inp["o_Wim"], f32)
    vbre = np.asarray(inp["v_bre"], f32)
    vbim = np.asarray(inp["v_bim"], f32)
    Mre_rows, Mim_rows, bv_rows = [], [], []
    for r in range(NCORES):
        for hh in range(HPC):
            h = r * HPC + hh
            hs = slice(h * D, (h + 1) * D)
            Mre_rows += [oWre[:, hs].T, -oWim[:, hs].T]
            Mim_rows += [oWim[:, hs].T, oWre[:, hs].T]
            bv_rows += [vbre[hs] - vbim[hs], vbre[hs] + vbim[hs]]
    M_re = np.concatenate(Mre_rows, axis=0)  # [2C, C]
    M_im = np.concatenate(Mim_rows, axis=0)
    bv_full = np.concatenate(bv_rows)  # [2C] — v bias in A2A row order
    o_bre = np.asarray(inp["o_bre"], f32)
    o_bim = np.asarray(inp["o_bim"], f32)
    # fold the v bias through the o-projection (softmax rows sum to 1)
    bo_re = (o_bre - o_bim) + M_re.T @ bv_full  # [C]
    bo_im = (o_bre + o_bim) + M_im.T @ bv_full
    bo_re = np.ascontiguousarray(bo_re.reshape(8, 128).T.astype(f32))  # [128, 8]
    bo_im = np.ascontiguousarray(bo_im.reshape(8, 128).T.astype(f32))
    m_re = np.ascontiguousarray(
        M_re.reshape(KT, 128, C).transpose(1, 0, 2).astype(BF)
    )  # [128, KT, C]
    m_im = np.ascontiguousarray(M_im.reshape(KT, 128, C).transpose(1, 0, 2).astype(BF))
    shared = dict(xc=xc, m_re=m_re, m_im=m_im, bo_re=bo_re, bo_im=bo_im)
    return shared, per_core


def _build_program():
    nc = bacc.Bacc("TRN2", target_bir_lowering=False, debug=False, num_devices=NCORES)

    # ---- DRAM I/O (all partition-major, contiguous per partition) ----
    xc_d = nc.dram_tensor("xc", [NCHT, 128, KT, TF], BF16, kind="ExternalInput")
    wq_d = nc.dram_tensor("wq", [128, HPC, KT, 128], BF16, kind="ExternalInput")
    wk_d = nc.dram_tensor("wk", [128, HPC, KT, 128], BF16, kind="ExternalInput")
    wv_d = nc.dram_tensor("wv", [128, KT, 2 * HPC * D], BF16, kind="ExternalInput")
    bq_d = nc.dram_tensor("bq", [128, HPC], F32, kind="ExternalInput")
    bk_d = nc.dram_tensor("bk", [128, HPC], F32, kind="ExternalInput")
    mre_d = nc.dram_tensor("m_re", [128, KT, C], BF16, kind="ExternalInput")
    mim_d = nc.dram_tensor("m_im", [128, KT, C], BF16, kind="ExternalInput")
    bore_d = nc.dram_tensor("bo_re", [128, 8], F32, kind="ExternalInput")
    boim_d = nc.dram_tensor("bo_im", [128, 8], F32, kind="ExternalInput")
    yout_d = nc.dram_tensor("yout", [2 * C, TSL], F32, kind="ExternalOutput")

    NCH = N // TF  # chunks per batch

    with (
        tile.TileContext(nc) as tc,
        nc.allow_low_precision(reason="bf16 compute; fp32 PSUM accumulation"),
    ):
        with tc.tile_pool(name="dram", bufs=1, space="DRAM") as dram:
            out_d = [
                dram.tile([NCORES, HPC * 128, TOKB], BF16, name=f"out_d{b}")
                for b in range(B)
            ]
            at_d = [
                dram.tile([NCORES, HPC * 128, TOKB], BF16, name=f"at_d{b}")
                for b in range(B)
            ]

            with tc.tile_pool(name="keep", bufs=1) as keep:
                # qc/kc/vt for both batches stay live through attention
                qc, kc, vt = {}, {}, {}
                with tc.tile_pool(name="qckc", bufs=1) as qckc:
                    for b in range(B):
                        for hh in range(HPC):
                            qc[b, hh] = qckc.tile(
                                [128, N], BF16, name=f"qc{b}{hh}", tag=f"qc{b}{hh}"
                            )
                            kc[b, hh] = qckc.tile(
                                [128, N], BF16, name=f"kc{b}{hh}", tag=f"kc{b}{hh}"
                            )
                            vt[b, hh] = qckc.tile(
                                [128, N // 128, 128],
                                BF16,
                                name=f"vt{b}{hh}",
                                tag=f"vt{b}{hh}",
                            )

                    ctxP = ExitStack()  # projection-phase pools
                    const = ctxP.enter_context(tc.tile_pool(name="const", bufs=1))
                    xp = ctxP.enter_context(tc.tile_pool(name="xp", bufs=2))
                    qk_ps = ctxP.enter_context(
                        tc.tile_pool(name="qk_ps", bufs=2, space="PSUM")
                    )
                    v_ps = ctxP.enter_context(
                        tc.tile_pool(name="v_ps", bufs=2, space="PSUM")
                    )

                    # ---- startup loads, split fine so the first q-chain can
                    # start as soon as xt0's first kt tiles + wq's first head
                    # land (three parallel queues; each sub-DMA completes its
                    # own dependency range) ----
                    xt0 = xp.tile([128, KT, TF], BF16, name="xt", tag="xt")
                    for i in range(4):
                        nc.sync.dma_start(
                            xt0[:, i * 4 : (i + 1) * 4, :], xc_d[0, :, i * 4 : (i + 1) * 4, :]
                        )
                    wq_sb = const.tile([128, HPC, KT, 128], BF16)
                    wk_sb = const.tile([128, HPC, KT, 128], BF16)
                    wv_sb = const.tile([128, KT, 2 * HPC * D], BF16)
                    for hh in range(HPC):
                        nc.scalar.dma_start(wq_sb[:, hh], wq_d[:, hh])
                        nc.gpsimd.dma_start(wk_sb[:, hh], wk_d[:, hh])
                    nc.sync.dma_start(wv_sb[:], wv_d[:])
                    bq_sb = keep.tile([128, HPC], F32)
                    bk_sb = keep.tile([128, HPC], F32)
                    nc.gpsimd.dma_start(bq_sb[:], bq_d[:])
                    nc.gpsimd.dma_start(bk_sb[:], bk_d[:])
                    bore_sb = keep.tile([128, 8], F32)
                    boim_sb = keep.tile([128, 8], F32)
                    nc.gpsimd.dma_start(bore_sb[:], bore_d[:])
                    nc.gpsimd.dma_start(boim_sb[:], boim_d[:])
                    ones_f = keep.tile([128, 128], F32)
                    nc.any.memset(ones_f[:], 1.0)
                    ones128 = keep.tile([128, 128], BF16)
                    nc.vector.tensor_copy(ones128[:], ones_f[:])

                    # warm the PE (HAM un-throttle) with throwaway matmuls
                    # while the first x/weight DMAs are in flight
                    warm = keep.tile([128, 512], BF16)
                    nc.any.memset(warm[:], 0.0)
                    with tc.tile_pool(name="wu_ps", bufs=1, space="PSUM") as wu_ps:
                        wups = wu_ps.tile([128, 512], F32)
                        for _ in range(24):
                            nc.tensor.matmul(
                                wups[:], ones128[:], warm[:], start=True, stop=True
                            )

                    # ---- projections: both batches ----
                    for chunk in range(NCHT):
                        b, ci = divmod(chunk, NCH)
                        if chunk == 0:
                            xt = xt0
                        else:
                            xt = xp.tile([128, KT, TF], BF16, name="xt", tag="xt")
                            nc.sync.dma_start(xt[:], xc_d[chunk])
                        csl = slice(ci * TF, ci * TF + TF)
                        for hh in range(HPC):
                            for w_sb, b_sb, dst in (
                                (wq_sb, bq_sb, qc[b, hh]),
                                (wk_sb, bk_sb, kc[b, hh]),
                            ):
                                ps = qk_ps.tile([128, TF], F32, name="qkps", tag="qkps")
                                for kt in range(KT):
                                    nc.tensor.matmul(
                                        ps[:],
                                        w_sb[:, hh, kt, :],
                                        xt[:, kt, :],
                                        start=(kt == 0),
                                        stop=(kt == KT - 1),
                                    )
                                nc.scalar.activation(
                                    dst[:, csl],
                                    ps[:],
                                    mybir.ActivationFunctionType.Identity,
                                    bias=b_sb[:, hh : hh + 1],
                                )
                        for m in range(TF // 128):
                            vp = v_ps.tile(
                                [128, 2 * HPC * D], F32, name="vps", tag="vps"
                            )
                            for kt in range(KT):
                                nc.tensor.matmul(
                                    vp[:],
                                    xt[:, kt, m * 128 : (m + 1) * 128],
                                    wv_sb[:, kt, :],
                                    start=(kt == 0),
                                    stop=(kt == KT - 1),
                                )
                            ktok = ci * (TF // 128) + m
                            for hh in range(HPC):
                                nc.vector.tensor_copy(
                                    vt[b, hh][:, ktok, :],
                                    vp[:, hh * 128 : (hh + 1) * 128],
                                )
                    ctxP.close()

                    # ---- o-projection loads (overlap attention) ----
                    ctxM = ExitStack()
                    mp = ctxM.enter_context(tc.tile_pool(name="mp", bufs=1))
                    mre_sb = mp.tile([128, KT, C], BF16, name="mre", tag="mre")
                    mim_sb = mp.tile([128, KT, C], BF16, name="mim", tag="mim")
                    nc.gpsimd.dma_start(mre_sb[:], mre_d[:])
                    nc.gpsimd.dma_start(mim_sb[:], mim_d[:])

                    # at_sb tiles live here so their loads can be issued on the
                    # gpsimd queue right after each collective completes
                    oc = ctxM.enter_context(tc.tile_pool(name="oc", bufs=1))
                    at_sb = {}
                    for b in range(B):
                        at_sb[b] = oc.tile(
                            [128, KT, TOKB], BF16, name=f"at{b}", tag=f"at{b}"
                        )

                    # ---- attention (full 8 PSUM banks available) ----
                    # software-pipelined: the next key-tile-pair's score
                    # matmuls are emitted BEFORE this pair's av/den matmuls so
                    # the (in-order) PE streams through Exp latency.
                    ctxA = ExitStack()
                    expp = ctxA.enter_context(tc.tile_pool(name="expp", bufs=3))
                    evp = ctxA.enter_context(tc.tile_pool(name="evp", bufs=3))
                    sc_ps = ctxA.enter_context(
                        tc.tile_pool(name="sc_ps", bufs=2, space="PSUM")
                    )
                    av_ps = ctxA.enter_context(
                        tc.tile_pool(name="av_ps", bufs=2, space="PSUM")
                    )
                    den_ps = ctxA.enter_context(
                        tc.tile_pool(name="den_ps", bufs=2, space="PSUM")
                    )
                    NKP = N // 256  # key-tile pairs
                    units = [
                        (b, hh, qt)
                        for b in range(B)
                        for hh in range(HPC)
                        for qt in range(N // 512)
                    ]
                    jobs = [(ui, ktp) for ui in range(len(units)) for ktp in range(NKP)]

                    def emit_sc(job):
                        ui, ktp = job
                        b, hh, qt = units[ui]
                        qsl = slice(qt * 512, qt * 512 + 512)
                        sc = sc_ps.tile([128, 1024], F32, name="scps", tag="scps")
                        for half in range(2):
                            kt = ktp * 2 + half
                            nc.tensor.matmul(
                                sc[:, half * 512 : half * 512 + 512],
                                kc[b, hh][:, kt * 128 : (kt + 1) * 128],
                                qc[b, hh][:, qsl],
                                start=True,
                                stop=True,
                            )
                        ex = expp.tile([128, 1024], BF16, name="ex", tag="ex")
                        nc.scalar.activation(
                            ex[:], sc[:], mybir.ActivationFunctionType.Exp, scale=0.125
                        )
                        # pair-sum the two key-tiles on DVE so the den matmul
                        # contracts once per PAIR (halves its PE cycles)
                        exs = expp.tile([128, 512], BF16, name="exs", tag="exs")
                        nc.vector.tensor_tensor(
                            exs[:], ex[:, 0:512], ex[:, 512:1024], mybir.AluOpType.add
                        )
                        return ex, exs

                    pend = {0: emit_sc(jobs[0])}  # job -> (ex, exs)
                    acc = {}
                    for j, (ui, ktp) in enumerate(jobs):
                        if j + 1 < len(jobs):
                            pend[j + 1] = emit_sc(jobs[j + 1])
                        b, hh, qt = units[ui]
                        ex, exs = pend.pop(j)
                        if ktp == 0:
                            acc[ui] = (
                                av_ps.tile([128, 512], F32, name="avps", tag="avps"),
                                den_ps.tile([128, 512], F32, name="denps", tag="denps"),
                            )
                        av, den = acc[ui]
                        for half in range(2):
                            kt = ktp * 2 + half
                            first = ktp == 0 and half == 0
                            last = ktp == NKP - 1 and half == 1
                            exh = ex[:, half * 512 : half * 512 + 512]
                            nc.tensor.matmul(
                                av[:], vt[b, hh][:, kt, :], exh, start=first, stop=last
                            )
                        nc.tensor.matmul(
                            den[:],
                            ones128[:],
                            exs[:],
                            start=(ktp == 0),
                            stop=(ktp == NKP - 1),
                        )
                        if ktp == NKP - 1:
                            del acc[ui]
                            rb = evp.tile([128, 512], F32, name="rb", tag="rb")
                            nc.vector.reciprocal(rb[:], den[:])
                            outc = evp.tile([128, 512], BF16, name="outc", tag="outc")
                            nc.vector.tensor_tensor(
                                outc[:], av[:], rb[:], mybir.AluOpType.mult
                            )
                            for s in range(2):
                                nc.sync.dma_start(
                                    out_d[b][qt * 2 + s, hh * 128 : (hh + 1) * 128, :],
                                    outc[:, s * 256 : s * 256 + 256],
                                )
                            if (ui + 1) % (len(units) // B) == 0:
                                # exchange this batch's attention outputs
                                # (overlaps the next batch's attention / the
                                # previous o-projection), then pull the result
                                # into SBUF on the same (idle) gpsimd queue
                                nc.gpsimd.collective_compute(
                                    "AllToAll",
                                    mybir.AluOpType.bypass,
                                    replica_groups=[list(range(NCORES))],
                                    ins=[out_d[b].opt()],
                                    outs=[at_d[b].opt()],
                                )
                                nc.gpsimd.dma_start(
                                    at_sb[b][:],
                                    at_d[b].rearrange(
                                        "r (hp p) t -> p (r hp) t", p=128
                                    ),
                                )
                    ctxA.close()

                    # ---- output projection: 256-token slice per batch ----
                    with (
                        tc.tile_pool(name="oev", bufs=3) as oev,
                        tc.tile_pool(name="o_ps", bufs=4, space="PSUM") as o_ps,
                    ):
                        for b in range(B):
                            for part, (m_sb, bo_sb) in enumerate(
                                ((mre_sb, bore_sb), (mim_sb, boim_sb))
                            ):
                                for ot in range(C // 128):
                                    ps = o_ps.tile(
                                        [128, TOKB], F32, name="ops", tag="ops"
                                    )
                                    for kt in range(KT):
                                        nc.tensor.matmul(
                                            ps[:],
                                            m_sb[:, kt, ot * 128 : (ot + 1) * 128],
                                            at_sb[b][:, kt, :],
                                            start=(kt == 0),
                                            stop=(kt == KT - 1),
                                        )
                                    y_sb = oev.tile(
                                        [128, TOKB], F32, name="y_sb", tag="y_sb"
                                    )
                                    nc.scalar.activation(
                                        y_sb[:],
                                        ps[:],
                                        mybir.ActivationFunctionType.Identity,
                                        bias=bo_sb[:, ot : ot + 1],
                                    )
                                    nc.sync.dma_start(
                                        yout_d[
                                            part * C + ot * 128 : part * C
                                            + (ot + 1) * 128,
                                            b * TOKB : (b + 1) * TOKB,
                                        ],
                                        y_sb[:],
                                    )
                    ctxM.close()
    nc.compile()
    return nc


_NC_CACHE = None


def _get_program():
    global _NC_CACHE
    if _NC_CACHE is None:
        _NC_CACHE = _build_program()
    return _NC_CACHE


def _run(inputs, trace=False, trace_kwargs=None):
    shared, per_core = _host_prep(inputs)
    nc = _get_program()
    in_maps = []
    for c in range(NCORES):
        d = per_core[c]
        in_maps.append(
            {
                "xc": shared["xc"],
                "wq": d["wq"],
                "wk": d["wk"],
                "wv": d["wv"],
                "bq": d["bq"],
                "bk": d["bk"],
                "m_re": shared["m_re"],
                "m_im": shared["m_im"],
                "bo_re": shared["bo_re"],
                "bo_im": shared["bo_im"],
            }
        )
    res = run_bass_kernel_spmd(
        nc, in_maps, list(range(NCORES)), trace=trace, **(trace_kwargs or {})
    )
    youts = [res.results[c]["yout"] for c in range(NCORES)]
    re = np.empty((C, B, N), dtype=np.float32)
    im = np.empty((C, B, N), dtype=np.float32)
    for c in range(NCORES):
        y = youts[c]
        tsl = slice(c * TOKB, (c + 1) * TOKB)
        re[:, 0, tsl] = y[:C, :TOKB]
        re[:, 1, tsl] = y[:C, TOKB:]
        im[:, 0, tsl] = y[C:, :TOKB]
        im[:, 1, tsl] = y[C:, TOKB:]
    out = np.stack([re.transpose(1, 2, 0), im.transpose(1, 2, 0)]).astype(np.float32)
    return out, res


def kernel(**inputs) -> np.ndarray:
    out, _ = _run(inputs, trace=False)
    return out


# revision 13
# speedup vs baseline: 1.1058x; 1.1058x over previous
"""Complex self-attention on 8 Trainium2 NeuronCores (Bass/Tile), v2.

Model (reference): complex linear q/k/v projections of (x_re, x_im), attention
scores = (Re(q)·Re(k) + Im(q)·Im(k))/sqrt(D), softmax, attn applied to Re(v)
and Im(v), complex output projection. B=2, N=2048, C=1024, H=16, D=64.

Sharding: heads tensor-parallel across 8 cores (2 heads/core, both batches).

v2 changes over the fp32r baseline (724µs):
  - bf16 everywhere (weights, x, q/k/v, exp-scores, attention outputs, o-proj
    matrices). PE rate is identical to fp32r (1 cycle/row) but DMA bytes halve,
    FWL (fast weight load) activates, and SBUF pressure halves. PSUM
    accumulation stays fp32.
  - Host pre-permutes every DRAM tensor into partition-major layout so each
    DMA is 128 large contiguous descriptors (the baseline's 512B-chunk gathers
    made the first x+weight load a 55µs stall).
  - All projections (both batches) run first; attention follows with the full
    8 PSUM banks: scores for a PAIR of key-tiles accumulate into one
    [128,1024] PSUM tile so a single Exp activation covers 1024 elements
    (halves ACT instruction overhead; scalar was pacing the attention loop).
  - den (softmax denominator) and av PSUM pools are double-buffered so the
    slow DVE reciprocal is off the PE critical path.
  - The AllToAll is split per batch ([8,256,256] bf16, ~1MB/core each):
    A2A(b0) overlaps batch-1 attention; A2A(b1) overlaps the b0 output
    projection. The baseline's single 4MB fp32 A2A exposed a 108µs stall.
"""

import sys

if "/opt/trn_rl_repo" not in sys.path:
    sys.path.insert(0, "/opt/trn_rl_repo")

from contextlib import ExitStack

import ml_dtypes
import numpy as np

import concourse.mybir as mybir
import concourse.tile as tile
from concourse import bacc
from concourse.bass_utils import run_bass_kernel_spmd

B, N, C = 2, 2048, 1024
H, D = 16, 64
T = B * N  # 4096 tokens total
NCORES = 8
HPC = H // NCORES  # 2 heads per core
TF = 512  # projection token-chunk (free dim)
NCHT = T // TF  # 8 chunks total over both batches
KT = 2 * C // 128  # 16 contraction tiles of 128 over [x_re; x_im]
TOKB = 256  # A2A token block (per batch, per core slice)
TSL = 512  # final per-core output token count (256 from each batch)
F32 = mybir.dt.float32
BF16 = mybir.dt.bfloat16
BF = ml_dtypes.bfloat16


def _host_prep(inp):
    """Pre-permute all tensors to partition-major bf16 for contiguous DMA."""
    f32 = np.float32
    x_re = np.asarray(inp["x_re"], f32).reshape(T, C)
    x_im = np.asarray(inp["x_im"], f32).reshape(T, C)
    xT2 = np.concatenate([x_re.T, x_im.T], axis=0)  # [2C, T]
    # [2C, T] -> [chunk, p, kt, t] so each 512-token chunk is one contiguous
    # 16KB block per partition
    xc = np.ascontiguousarray(
        xT2.reshape(KT, 128, NCHT, TF).transpose(2, 1, 0, 3).astype(BF)
    )

    per_core = []
    for c in range(NCORES):
        d = {}
        h0 = c * HPC
        ch = slice(h0 * D, (h0 + HPC) * D)
        for nm in ("q", "k", "v"):
            Wre = np.asarray(inp[f"{nm}_Wre"], f32)[ch]  # [128, C]
            Wim = np.asarray(inp[f"{nm}_Wim"], f32)[ch]
            bre = np.asarray(inp[f"{nm}_bre"], f32)[ch]
            bim = np.asarray(inp[f"{nm}_bim"], f32)[ch]
            Ws, bs = [], []
            for hh in range(HPC):
                hs = slice(hh * D, (hh + 1) * D)
                wr = np.concatenate([Wre[hs].T, -Wim[hs].T], axis=0)  # [2C, 64]
                wi = np.concatenate([Wim[hs].T, Wre[hs].T], axis=0)
                Ws.append(np.concatenate([wr, wi], axis=1))  # [2C, 128]
                bs.append(np.concatenate([bre[hs] - bim[hs], bre[hs] + bim[hs]]))
            if nm == "v":
                wv = np.concatenate(Ws, axis=1)  # [2C, 256]
                d["wv"] = np.ascontiguousarray(
                    wv.reshape(KT, 128, 2 * HPC * D).transpose(1, 0, 2).astype(BF)
                )  # [128, KT, 256]
            else:
                w = np.stack(Ws)  # [HPC, 2C, 128]
                d[f"w{nm}"] = np.ascontiguousarray(
                    w.reshape(HPC, KT, 128, 128).transpose(2, 0, 1, 3).astype(BF)
                )  # [128, HPC, KT, 128]
                d[f"b{nm}"] = np.ascontiguousarray(np.stack(bs, axis=1))  # [128, HPC]
        per_core.append(d)

    # o-projection combined matrices, rows ordered to match the A2A result:
    # rank r, then per rank [h0:out_r(64), h0:out_i(64), h1:out_r(64), h1:out_i(64)]
    oWre = np.asarray(inp["o_Wre"], f32)
    oWim = np.asarray(inp["o_Wim"], f32)
    vbre = np.asarray(inp["v_bre"], f32)
    vbim = np.asarray(inp["v_bim"], f32)
    Mre_rows, Mim_rows, bv_rows = [], [], []
    for r in range(NCORES):
        for hh in range(HPC):
            h = r * HPC + hh
            hs = slice(h * D, (h + 1) * D)
            Mre_rows += [oWre[:, hs].T, -oWim[:, hs].T]
            Mim_rows += [oWim[:, hs].T, oWre[:, hs].T]
            bv_rows += [vbre[hs] - vbim[hs], vbre[hs] + vbim[hs]]
    M_re = np.concatenate(Mre_rows, axis=0)  # [2C, C]
    M_im = np.concatenate(Mim_rows, axis=0)
    bv_full = np.concatenate(bv_rows)  # [2C] — v bias in A2A row order
    o_bre = np.asarray(inp["o_bre"], f32)
    o_bim = np.asarray(inp["o_bim"], f32)
    # fold the v bias through the o-projection (softmax rows sum to 1)
    bo_re = (o_bre - o_bim) + M_re.T @ bv_full  # [C]
    bo_im = (o_bre + o_bim) + M_im.T @ bv_full
    bo_re = np.ascontiguousarray(bo_re.reshape(8, 128).T.astype(f32))  # [128, 8]
    bo_im = np.ascontiguousarray(bo_im.reshape(8, 128).T.astype(f32))
    m_re = np.ascontiguousarray(
        M_re.reshape(KT, 128, C).transpose(1, 0, 2).astype(BF)
    )  # [128, KT, C]
    m_im = np.ascontiguousarray(M_im.reshape(KT, 128, C).transpose(1, 0, 2).astype(BF))
    shared = dict(xc=xc, m_re=m_re, m_im=m_im, bo_re=bo_re, bo_im=bo_im)
    return shared, per_core


def _build_program():
    nc = bacc.Bacc("TRN2", target_bir_lowering=False, debug=False, num_devices=NCORES)

    # ---- DRAM I/O (all partition-major, contiguous per partition) ----
    xc_d = nc.dram_tensor("xc", [NCHT, 128, KT, TF], BF16, kind="ExternalInput")
    wq_d = nc.dram_tensor("wq", [128, HPC, KT, 128], BF16, kind="ExternalInput")
    wk_d = nc.dram_tensor("wk", [128, HPC, KT, 128], BF16, kind="ExternalInput")
    wv_d = nc.dram_tensor("wv", [128, KT, 2 * HPC * D], BF16, kind="ExternalInput")
    bq_d = nc.dram_tensor("bq", [128, HPC], F32, kind="ExternalInput")
    bk_d = nc.dram_tensor("bk", [128, HPC], F32, kind="ExternalInput")
    mre_d = nc.dram_tensor("m_re", [128, KT, C], BF16, kind="ExternalInput")
    mim_d = nc.dram_tensor("m_im", [128, KT, C], BF16, kind="ExternalInput")
    bore_d = nc.dram_tensor("bo_re", [128, 8], F32, kind="ExternalInput")
    boim_d = nc.dram_tensor("bo_im", [128, 8], F32, kind="ExternalInput")
    yout_d = nc.dram_tensor("yout", [2 * C, TSL], F32, kind="ExternalOutput")

    NCH = N // TF  # chunks per batch

    with (
        tile.TileContext(nc) as tc,
        nc.allow_low_precision(reason="bf16 compute; fp32 PSUM accumulation"),
    ):
        with tc.tile_pool(name="dram", bufs=1, space="DRAM") as dram:
            out_d = [
                dram.tile([NCORES, HPC * 128, TOKB], BF16, name=f"out_d{b}")
                for b in range(B)
            ]
            at_d = [
                dram.tile([NCORES, HPC * 128, TOKB], BF16, name=f"at_d{b}")
                for b in range(B)
            ]

            with tc.tile_pool(name="keep", bufs=1) as keep:
                # qc/kc/vt for both batches stay live through attention
                qc, kc, vt = {}, {}, {}
                with tc.tile_pool(name="qckc", bufs=1) as qckc:
                    for b in range(B):
                        for hh in range(HPC):
                            qc[b, hh] = qckc.tile(
                                [128, N], BF16, name=f"qc{b}{hh}", tag=f"qc{b}{hh}"
                            )
                            kc[b, hh] = qckc.tile(
                                [128, N], BF16, name=f"kc{b}{hh}", tag=f"kc{b}{hh}"
                            )
                            vt[b, hh] = qckc.tile(
                                [128, N // 128, 128],
                                BF16,
                                name=f"vt{b}{hh}",
                                tag=f"vt{b}{hh}",
                            )

                    ctxP = ExitStack()  # projection-phase pools
                    const = ctxP.enter_context(tc.tile_pool(name="const", bufs=1))
                    xp = ctxP.enter_context(tc.tile_pool(name="xp", bufs=2))
                    qk_ps = ctxP.enter_context(
                        tc.tile_pool(name="qk_ps", bufs=2, space="PSUM")
                    )
                    v_ps = ctxP.enter_context(
                        tc.tile_pool(name="v_ps", bufs=2, space="PSUM")
                    )

                    # ---- startup loads, split fine so the first q-chain can
                    # start as soon as xt0's first kt tiles + wq's first head
                    # land (three parallel queues; each sub-DMA completes its
                    # own dependency range) ----
                    xt0 = xp.tile([128, KT, TF], BF16, name="xt", tag="xt")
                    for i in range(4):
                        nc.sync.dma_start(
                            xt0[:, i * 4 : (i + 1) * 4, :], xc_d[0, :, i * 4 : (i + 1) * 4, :]
                        )
                    # weights ordered by first use across the three queues:
                    # q(hh0) first, then k(hh0), q(hh1) (on the fast sync/HWDGE
                    # queue behind xt0), k(hh1), then v
                    wq_sb = const.tile([128, HPC, KT, 128], BF16)
                    wk_sb = const.tile([128, HPC, KT, 128], BF16)
                    wv_sb = const.tile([128, KT, 2 * HPC * D], BF16)
                    nc.scalar.dma_start(wq_sb[:, 0], wq_d[:, 0])
                    nc.gpsimd.dma_start(wk_sb[:, 0], wk_d[:, 0])
                    nc.sync.dma_start(wq_sb[:, 1], wq_d[:, 1])
                    nc.scalar.dma_start(wk_sb[:, 1], wk_d[:, 1])
                    nc.sync.dma_start(wv_sb[:], wv_d[:])
                    bq_sb = keep.tile([128, HPC], F32)
                    bk_sb = keep.tile([128, HPC], F32)
                    nc.gpsimd.dma_start(bq_sb[:], bq_d[:])
                    nc.gpsimd.dma_start(bk_sb[:], bk_d[:])
                    bore_sb = keep.tile([128, 8], F32)
                    boim_sb = keep.tile([128, 8], F32)
                    nc.gpsimd.dma_start(bore_sb[:], bore_d[:])
                    nc.gpsimd.dma_start(boim_sb[:], boim_d[:])
                    ones_f = keep.tile([128, 128], F32)
                    nc.any.memset(ones_f[:], 1.0)
                    ones128 = keep.tile([128, 128], BF16)
                    nc.vector.tensor_copy(ones128[:], ones_f[:])

                    # warm the PE (HAM un-throttle) with throwaway matmuls
                    # while the first x/weight DMAs are in flight
                    warm = keep.tile([128, 512], BF16)
                    nc.any.memset(warm[:], 0.0)
                    with tc.tile_pool(name="wu_ps", bufs=1, space="PSUM") as wu_ps:
                        wups = wu_ps.tile([128, 512], F32)
                        for _ in range(24):
                            nc.tensor.matmul(
                                wups[:], ones128[:], warm[:], start=True, stop=True
                            )

                    # ---- projections: both batches ----
                    for chunk in range(NCHT):
                        b, ci = divmod(chunk, NCH)
                        if chunk == 0:
                            xt = xt0
                        else:
                            xt = xp.tile([128, KT, TF], BF16, name="xt", tag="xt")
                            nc.sync.dma_start(xt[:], xc_d[chunk])
                        csl = slice(ci * TF, ci * TF + TF)
                        for hh in range(HPC):
                            for w_sb, b_sb, dst in (
                                (wq_sb, bq_sb, qc[b, hh]),
                                (wk_sb, bk_sb, kc[b, hh]),
                            ):
                                ps = qk_ps.tile([128, TF], F32, name="qkps", tag="qkps")
                                for kt in range(KT):
                                    nc.tensor.matmul(
                                        ps[:],
                                        w_sb[:, hh, kt, :],
                                        xt[:, kt, :],
                                        start=(kt == 0),
                                        stop=(kt == KT - 1),
                                    )
                                nc.scalar.activation(
                                    dst[:, csl],
                                    ps[:],
                                    mybir.ActivationFunctionType.Identity,
                                    bias=b_sb[:, hh : hh + 1],
                                )
                        for m in range(TF // 128):
                            vp = v_ps.tile(
                                [128, 2 * HPC * D], F32, name="vps", tag="vps"
                            )
                            for kt in range(KT):
                                nc.tensor.matmul(
                                    vp[:],
                                    xt[:, kt, m * 128 : (m + 1) * 128],
                                    wv_sb[:, kt, :],
                                    start=(kt == 0),
                                    stop=(kt == KT - 1),
                                )
                            ktok = ci * (TF // 128) + m
                            for hh in range(HPC):
                                nc.vector.tensor_copy(
                                    vt[b, hh][:, ktok, :],
                                    vp[:, hh * 128 : (hh + 1) * 128],
                                )
                    ctxP.close()

                    # ---- o-projection loads (overlap attention) ----
                    ctxM = ExitStack()
                    mp = ctxM.enter_context(tc.tile_pool(name="mp", bufs=1))
                    mre_sb = mp.tile([128, KT, C], BF16, name="mre", tag="mre")
                    mim_sb = mp.tile([128, KT, C], BF16, name="mim", tag="mim")
                    nc.gpsimd.dma_start(mre_sb[:], mre_d[:])
                    nc.gpsimd.dma_start(mim_sb[:], mim_d[:])

                    # at_sb tiles live here so their loads can be issued on the
                    # gpsimd queue right after each collective completes
                    oc = ctxM.enter_context(tc.tile_pool(name="oc", bufs=1))
                    at_sb = {}
                    for b in range(B):
                        at_sb[b] = oc.tile(
                            [128, KT, TOKB], BF16, name=f"at{b}", tag=f"at{b}"
                        )

                    # ---- attention (full 8 PSUM banks available) ----
                    # software-pipelined: the next key-tile-pair's score
                    # matmuls are emitted BEFORE this pair's av/den matmuls so
                    # the (in-order) PE streams through Exp latency.
                    ctxA = ExitStack()
                    expp = ctxA.enter_context(tc.tile_pool(name="expp", bufs=3))
                    evp = ctxA.enter_context(tc.tile_pool(name="evp", bufs=3))
                    sc_ps = ctxA.enter_context(
                        tc.tile_pool(name="sc_ps", bufs=2, space="PSUM")
                    )
                    av_ps = ctxA.enter_context(
                        tc.tile_pool(name="av_ps", bufs=2, space="PSUM")
                    )
                    den_ps = ctxA.enter_context(
                        tc.tile_pool(name="den_ps", bufs=2, space="PSUM")
                    )
                    NKP = N // 256  # key-tile pairs
                    units = [
                        (b, hh, qt)
                        for b in range(B)
                        for hh in range(HPC)
                        for qt in range(N // 512)
                    ]
                    jobs = [(ui, ktp) for ui in range(len(units)) for ktp in range(NKP)]

                    def emit_sc(job):
                        ui, ktp = job
                        b, hh, qt = units[ui]
                        qsl = slice(qt * 512, qt * 512 + 512)
                        sc = sc_ps.tile([128, 1024], F32, name="scps", tag="scps")
                        for half in range(2):
                            kt = ktp * 2 + half
                            nc.tensor.matmul(
                                sc[:, half * 512 : half * 512 + 512],
                                kc[b, hh][:, kt * 128 : (kt + 1) * 128],
                                qc[b, hh][:, qsl],
                                start=True,
                                stop=True,
                            )
                        ex = expp.tile([128, 1024], BF16, name="ex", tag="ex")
                        nc.scalar.activation(
                            ex[:], sc[:], mybir.ActivationFunctionType.Exp, scale=0.125
                        )
                        # pair-sum the two key-tiles on DVE so the den matmul
                        # contracts once per PAIR (halves its PE cycles)
                        exs = expp.tile([128, 512], BF16, name="exs", tag="exs")
                        nc.vector.tensor_tensor(
                            exs[:], ex[:, 0:512], ex[:, 512:1024], mybir.AluOpType.add
                        )
                        return ex, exs

                    pend = {0: emit_sc(jobs[0])}  # job -> (ex, exs)
                    acc = {}
                    for j, (ui, ktp) in enumerate(jobs):
                        if j + 1 < len(jobs):
                            pend[j + 1] = emit_sc(jobs[j + 1])
                        b, hh, qt = units[ui]
                        ex, exs = pend.pop(j)
                        if ktp == 0:
                            acc[ui] = (
                                av_ps.tile([128, 512], F32, name="avps", tag="avps"),
                                den_ps.tile([128, 512], F32, name="denps", tag="denps"),
                            )
                        av, den = acc[ui]
                        for half in range(2):
                            kt = ktp * 2 + half
                            first = ktp == 0 and half == 0
                            last = ktp == NKP - 1 and half == 1
                            exh = ex[:, half * 512 : half * 512 + 512]
                            nc.tensor.matmul(
                                av[:], vt[b, hh][:, kt, :], exh, start=first, stop=last
                            )
                        nc.tensor.matmul(
                            den[:],
                            ones128[:],
                            exs[:],
                            start=(ktp == 0),
                            stop=(ktp == NKP - 1),
                        )
                        if ktp == NKP - 1:
                            del acc[ui]
                            rb = evp.tile([128, 512], F32, name="rb", tag="rb")
                            # den is a sum of exps in [~1, ~1e5]: no edge
                            # cases, and ~18 correct bits beats the bf16
                            # pipeline noise. 5x faster than reciprocal() so
                            # it doesn't head-of-line-block the DVE queue.
                            nc.vector.reciprocal_approx_fast(rb[:], den[:])
                            outc = evp.tile([128, 512], BF16, name="outc", tag="outc")
                            nc.vector.tensor_tensor(
                                outc[:], av[:], rb[:], mybir.AluOpType.mult
                            )
                            for s in range(2):
                                nc.sync.dma_start(
                                    out_d[b][qt * 2 + s, hh * 128 : (hh + 1) * 128, :],
                                    outc[:, s * 256 : s * 256 + 256],
                                )
                            if (ui + 1) % (len(units) // B) == 0:
                                # exchange this batch's attention outputs
                                # (overlaps the next batch's attention / the
                                # previous o-projection), then pull the result
                                # into SBUF on the same (idle) gpsimd queue
                                nc.gpsimd.collective_compute(
                                    "AllToAll",
                                    mybir.AluOpType.bypass,
                                    replica_groups=[list(range(NCORES))],
                                    ins=[out_d[b].opt()],
                                    outs=[at_d[b].opt()],
                                )
                                nc.gpsimd.dma_start(
                                    at_sb[b][:],
                                    at_d[b].rearrange(
                                        "r (hp p) t -> p (r hp) t", p=128
                                    ),
                                )
                    ctxA.close()

                    # ---- output projection: 256-token slice per batch ----
                    with (
                        tc.tile_pool(name="oev", bufs=3) as oev,
                        tc.tile_pool(name="o_ps", bufs=4, space="PSUM") as o_ps,
                    ):
                        for b in range(B):
                            for part, (m_sb, bo_sb) in enumerate(
                                ((mre_sb, bore_sb), (mim_sb, boim_sb))
                            ):
                                for ot in range(C // 128):
                                    ps = o_ps.tile(
                                        [128, TOKB], F32, name="ops", tag="ops"
                                    )
                                    for kt in range(KT):
                                        nc.tensor.matmul(
                                            ps[:],
                                            m_sb[:, kt, ot * 128 : (ot + 1) * 128],
                                            at_sb[b][:, kt, :],
                                            start=(kt == 0),
                                            stop=(kt == KT - 1),
                                        )
                                    y_sb = oev.tile(
                                        [128, TOKB], F32, name="y_sb", tag="y_sb"
                                    )
                                    nc.scalar.activation(
                                        y_sb[:],
                                        ps[:],
                                        mybir.ActivationFunctionType.Identity,
                                        bias=bo_sb[:, ot : ot + 1],
                                    )
                                    nc.sync.dma_start(
                                        yout_d[
                                            part * C + ot * 128 : part * C
                                            + (ot + 1) * 128,
                                            b * TOKB : (b + 1) * TOKB,
                                        ],
                                        y_sb[:],
                                    )
                    ctxM.close()
    nc.compile()
    return nc


_NC_CACHE = None


def _get_program():
    global _NC_CACHE
    if _NC_CACHE is None:
        _NC_CACHE = _build_program()
    return _NC_CACHE


def _run(inputs, trace=False, trace_kwargs=None):
    shared, per_core = _host_prep(inputs)
    nc = _get_program()
    in_maps = []
    for c in range(NCORES):
        d = per_core[c]
        in_maps.append(
            {
                "xc": shared["xc"],
                "wq": d["wq"],
                "wk": d["wk"],
                "wv": d["wv"],
                "bq": d["bq"],
                "bk": d["bk"],
                "m_re": shared["m_re"],
                "m_im": shared["m_im"],
                "bo_re": shared["bo_re"],
                "bo_im": shared["bo_im"],
            }
        )
    res = run_bass_kernel_spmd(
        nc, in_maps, list(range(NCORES)), trace=trace, **(trace_kwargs or {})
    )
    youts = [res.results[c]["yout"] for c in range(NCORES)]
    re = np.empty((C, B, N), dtype=np.float32)
    im = np.empty((C, B, N), dtype=np.float32)
    for c in range(NCORES):
        y = youts[c]
        tsl = slice(c * TOKB, (c + 1) * TOKB)
        re[:, 0, tsl] = y[:C, :TOKB]
        re[:, 1, tsl] = y[:C, TOKB:]
        im[:, 0, tsl] = y[C:, :TOKB]
        im[:, 1, tsl] = y[C:, TOKB:]
    out = np.stack([re.transpose(1, 2, 0), im.transpose(1, 2, 0)]).astype(np.float32)
    return out, res


def kernel(**inputs) -> np.ndarray:
    out, _ = _run(inputs, trace=False)
    return out


# revision 23
# speedup vs baseline: 1.1738x; 1.0616x over previous
"""Complex self-attention on 8 Trainium2 NeuronCores (Bass/Tile), v6.

Model (reference): complex linear q/k/v projections of (x_re, x_im), attention
scores = (Re(q)·Re(k) + Im(q)·Im(k))/sqrt(D), softmax, attn applied to Re(v)
and Im(v), complex output projection. B=2, N=2048, C=1024, H=16, D=64.

Sharding: heads tensor-parallel across 8 cores (2 heads/core, both batches).

Structure (all engines bf16, fp32 PSUM):
  - Host pre-permutes every DRAM tensor partition-major for contiguous DMA.
  - q/k and o projections use the 3-multiplication (Karatsuba) complex
    form: P1 = x_re@Wre, P2 = x_im@Wim, P3 = (x_re+x_im)@(Wre+Wim);
    re = P1-P2, im = P3-P1-P2 (combines on the DVE, biases fused via
    scalar_tensor_tensor). 25% fewer PE cycles than the stacked form.
  - The k projection bias is DROPPED: (q+bq)·(k+bk) = (q+bq)·k + const(n),
    and per-query constants cancel in softmax. Exactly equivalent math.
  - v keeps the stacked 4-mult form (produces the [keys, re|im] layout the
    attention matmuls need; its combines would be overhead-dominated).
  - Attention is software-pipelined one key-tile-pair ahead so the in-order
    PE streams through the Exp latency; exp runs on [128,1024] PSUM tiles;
    the softmax denominator contracts DVE pair-sums (half the den matmuls)
    and uses the fast approximate reciprocal.
  - Per-batch AllToAll ([8,256,256] bf16) with re/im channel blocks per rank;
    A2A(b0) overlaps batch-1 attention, A2A(b1) overlaps the b0 output
    projection; at tiles load on the gpsimd queue right after each A2A.
"""

import sys

if "/opt/trn_rl_repo" not in sys.path:
    sys.path.insert(0, "/opt/trn_rl_repo")

from contextlib import ExitStack

import ml_dtypes
import numpy as np

import concourse.mybir as mybir
import concourse.tile as tile
from concourse import bacc
from concourse.bass_utils import run_bass_kernel_spmd

B, N, C = 2, 2048, 1024
H, D = 16, 64
T = B * N  # 4096 tokens total
NCORES = 8
HPC = H // NCORES  # 2 heads per core
TF = 512  # projection token-chunk (free dim)
NCHT = T // TF  # 8 chunks total over both batches
KT = 2 * C // 128  # 16 contraction tiles of 128 over [x_re; x_im]
KH = KT // 2  # 8 tiles over one of x_re / x_im
TOKB = 256  # A2A token block (per batch, per core slice)
TSL = 512  # final per-core output token count (256 from each batch)
F32 = mybir.dt.float32
BF16 = mybir.dt.bfloat16
BF = ml_dtypes.bfloat16
ADD = mybir.AluOpType.add
SUB = mybir.AluOpType.subtract


def _host_prep(inp):
    """Pre-permute all tensors to partition-major bf16 for contiguous DMA."""
    f32 = np.float32
    x_re = np.asarray(inp["x_re"], f32).reshape(T, C)
    x_im = np.asarray(inp["x_im"], f32).reshape(T, C)
    xT2 = np.concatenate([x_re.T, x_im.T], axis=0)  # [2C, T]
    # [2C, T] -> [chunk, p, kt, t]: one contiguous 16KB block per partition
    xc = np.ascontiguousarray(
        xT2.reshape(KT, 128, NCHT, TF).transpose(2, 1, 0, 3).astype(BF)
    )

    def three_mult(Wre, Wim):
        # [C, 3, KH, cols]-style stack: slot 0 = Wre.T, 1 = Wim.T, 2 = sum.T
        w3 = np.stack([Wre.T, Wim.T, (Wre + Wim).T])  # [3, C, cols]
        cols = w3.shape[2]
        return np.ascontiguousarray(
            w3.reshape(3, KH, 128, cols).transpose(2, 0, 1, 3).astype(BF)
        )  # [128, 3, KH, cols]

    per_core = []
    for c in range(NCORES):
        d = {}
        h0 = c * HPC
        ch = slice(h0 * D, (h0 + HPC) * D)
        # q/k: 3M weights, cols = [h0 dims 64 | h1 dims 64]
        for nm in ("q", "k"):
            Wre = np.asarray(inp[f"{nm}_Wre"], f32)[ch]  # [128, C]
            Wim = np.asarray(inp[f"{nm}_Wim"], f32)[ch]
            d[f"w{nm}"] = three_mult(Wre, Wim)  # [128, 3, KH, 128]
        bre = np.asarray(inp["q_bre"], f32)[ch]
        bim = np.asarray(inp["q_bim"], f32)[ch]
        # [64, 4]: cols = (hh, re/im) so every slice starts at partition 0
        bq = np.stack(
            [
                np.stack([(bre - bim)[hh * D : (hh + 1) * D], (bre + bim)[hh * D : (hh + 1) * D]], axis=1)
                for hh in range(HPC)
            ],
            axis=1,
        ).reshape(D, 2 * HPC)
        d["bq"] = np.ascontiguousarray(bq)
        # v: stacked 4-mult form, [2C, 256] cols = per-head [re64 im64]
        Wre = np.asarray(inp["v_Wre"], f32)[ch]
        Wim = np.asarray(inp["v_Wim"], f32)[ch]
        Ws = []
        for hh in range(HPC):
            hs = slice(hh * D, (hh + 1) * D)
            wr = np.concatenate([Wre[hs].T, -Wim[hs].T], axis=0)  # [2C, 64]
            wi = np.concatenate([Wim[hs].T, Wre[hs].T], axis=0)
            Ws.append(np.concatenate([wr, wi], axis=1))  # [2C, 128]
        wv = np.concatenate(Ws, axis=1)  # [2C, 256]
        d["wv"] = np.ascontiguousarray(
            wv.reshape(KT, 128, 2 * HPC * D).transpose(1, 0, 2).astype(BF)
        )  # [128, KT, 256]
        per_core.append(d)

    # o-projection 3M matrices. The A2A delivers, per source rank r, a
    # 128-row re block then a 128-row im block; re rows across ranks span the
    # v-output channels in natural order, so M1/M2/M3 are just the transposed
    # (oWre, oWim, sum) matrices tiled [128, 3, KH, C].
    oWre = np.asarray(inp["o_Wre"], f32)
    oWim = np.asarray(inp["o_Wim"], f32)
    m3 = three_mult(oWre, oWim)  # [128, 3, 8, 1024]
    vbre = np.asarray(inp["v_bre"], f32)
    vbim = np.asarray(inp["v_bim"], f32)
    o_bre = np.asarray(inp["o_bre"], f32)
    o_bim = np.asarray(inp["o_bim"], f32)
    # fold the v bias through the o-projection (softmax rows sum to 1)
    bo_re = (o_bre - o_bim) + oWre @ (vbre - vbim) - oWim @ (vbre + vbim)
    bo_im = (o_bre + o_bim) + oWim @ (vbre - vbim) + oWre @ (vbre + vbim)
    bo_re = np.ascontiguousarray(bo_re.reshape(8, 128).T.astype(f32))  # [128, 8]
    bo_im = np.ascontiguousarray(bo_im.reshape(8, 128).T.astype(f32))
    shared = dict(xc=xc, m3=m3, bo_re=bo_re, bo_im=bo_im)
    return shared, per_core


def _build_program():
    nc = bacc.Bacc("TRN2", target_bir_lowering=False, debug=False, num_devices=NCORES)

    # ---- DRAM I/O (all partition-major, contiguous per partition) ----
    xc_d = nc.dram_tensor("xc", [NCHT, 128, KT, TF], BF16, kind="ExternalInput")
    wq_d = nc.dram_tensor("wq", [128, 3, KH, 128], BF16, kind="ExternalInput")
    wk_d = nc.dram_tensor("wk", [128, 3, KH, 128], BF16, kind="ExternalInput")
    wv_d = nc.dram_tensor("wv", [128, KT, 2 * HPC * D], BF16, kind="ExternalInput")
    bq_d = nc.dram_tensor("bq", [64, 2 * HPC], F32, kind="ExternalInput")
    m3_d = nc.dram_tensor("m3", [128, 3, KH, C], BF16, kind="ExternalInput")
    bore_d = nc.dram_tensor("bo_re", [128, 8], F32, kind="ExternalInput")
    boim_d = nc.dram_tensor("bo_im", [128, 8], F32, kind="ExternalInput")
    yout_d = nc.dram_tensor("yout", [2 * C, TSL], F32, kind="ExternalOutput")

    NCH = N // TF  # chunks per batch

    with (
        tile.TileContext(nc) as tc,
        nc.allow_low_precision(reason="bf16 compute; fp32 PSUM accumulation"),
    ):
        with tc.tile_pool(name="dram", bufs=1, space="DRAM") as dram:
            out_d = [
                dram.tile([NCORES, 2 * HPC * D, TOKB], BF16, name=f"out_d{b}")
                for b in range(B)
            ]
            at_d = [
                dram.tile([NCORES, 2 * HPC * D, TOKB], BF16, name=f"at_d{b}")
                for b in range(B)
            ]

            with tc.tile_pool(name="keep", bufs=1) as keep:
                qc, kc, vt = {}, {}, {}
                with tc.tile_pool(name="qckc", bufs=1) as qckc:
                    for b in range(B):
                        for hh in range(HPC):
                            qc[b, hh] = qckc.tile(
                                [128, N], BF16, name=f"qc{b}{hh}", tag=f"qc{b}{hh}"
                            )
                            kc[b, hh] = qckc.tile(
                                [128, N], BF16, name=f"kc{b}{hh}", tag=f"kc{b}{hh}"
                            )
                            vt[b, hh] = qckc.tile(
                                [128, N // 128, 128],
                                BF16,
                                name=f"vt{b}{hh}",
                                tag=f"vt{b}{hh}",
                            )

                    ctxP = ExitStack()  # projection-phase pools
                    const = ctxP.enter_context(tc.tile_pool(name="const", bufs=1))
                    xp = ctxP.enter_context(tc.tile_pool(name="xp", bufs=2))
                    xsp = ctxP.enter_context(tc.tile_pool(name="xsp", bufs=2))
                    stg = ctxP.enter_context(tc.tile_pool(name="stg", bufs=3))

                    # ---- startup loads, split fine so the first P1 chain can
                    # start once xt0's re-half + wq slot 0 land ----
                    xt0 = xp.tile([128, KT, TF], BF16, name="xt", tag="xt")
                    for i in range(4):
                        nc.sync.dma_start(
                            xt0[:, i * 4 : (i + 1) * 4, :],
                            xc_d[0, :, i * 4 : (i + 1) * 4, :],
                        )
                    wq_sb = const.tile([128, 3, KH, 128], BF16)
                    wk_sb = const.tile([128, 3, KH, 128], BF16)
                    wv_sb = const.tile([128, KT, 2 * HPC * D], BF16)
                    nc.scalar.dma_start(wq_sb[:, 0:2], wq_d[:, 0:2])
                    nc.gpsimd.dma_start(wk_sb[:, 0:2], wk_d[:, 0:2])
                    nc.sync.dma_start(wq_sb[:, 2:3], wq_d[:, 2:3])
                    nc.scalar.dma_start(wk_sb[:, 2:3], wk_d[:, 2:3])
                    nc.sync.dma_start(wv_sb[:], wv_d[:])
                    bq_sb = keep.tile([64, 2 * HPC], F32)
                    nc.gpsimd.dma_start(bq_sb[:], bq_d[:])
                    bore_sb = keep.tile([128, 8], F32)
                    boim_sb = keep.tile([128, 8], F32)
                    nc.gpsimd.dma_start(bore_sb[:], bore_d[:])
                    nc.gpsimd.dma_start(boim_sb[:], boim_d[:])
                    ones_f = keep.tile([128, 128], F32)
                    nc.any.memset(ones_f[:], 1.0)
                    ones128 = keep.tile([128, 128], BF16)
                    nc.vector.tensor_copy(ones128[:], ones_f[:])

                    # warm the PE (HAM un-throttle) while startup DMAs fly
                    warm = keep.tile([128, 512], BF16)
                    nc.any.memset(warm[:], 0.0)
                    with tc.tile_pool(name="wu_ps", bufs=1, space="PSUM") as wu_ps:
                        wups = wu_ps.tile([128, 512], F32)
                        for _ in range(24):
                            nc.tensor.matmul(
                                wups[:], ones128[:], warm[:], start=True, stop=True
                            )
                    p3_ps = ctxP.enter_context(
                        tc.tile_pool(name="p3_ps", bufs=2, space="PSUM")
                    )
                    v_ps = ctxP.enter_context(
                        tc.tile_pool(name="v_ps", bufs=2, space="PSUM")
                    )

                    # ---- projections: both batches ----
                    for chunk in range(NCHT):
                        b, ci = divmod(chunk, NCH)
                        if chunk == 0:
                            xt = xt0
                        else:
                            xt = xp.tile([128, KT, TF], BF16, name="xt", tag="xt")
                            nc.sync.dma_start(xt[:], xc_d[chunk])
                        xts = xsp.tile([128, KH, TF], BF16, name="xts", tag="xts")
                        nc.vector.tensor_tensor(
                            xts[:], xt[:, 0:KH, :], xt[:, KH:KT, :], ADD
                        )
                        csl = slice(ci * TF, ci * TF + TF)
                        # q/k 3M chains + DVE combines
                        for w_sb, dst, is_q in ((wq_sb, qc, True), (wk_sb, kc, False)):
                            ps3 = p3_ps.tile([128, 3, TF], F32, name="p3", tag="p3")
                            for s, src in (
                                (0, xt[:, 0:KH, :]),
                                (1, xt[:, KH:KT, :]),
                                (2, xts[:]),
                            ):
                                for kt in range(KH):
                                    nc.tensor.matmul(
                                        ps3[:, s, :],
                                        w_sb[:, s, kt, :],
                                        src[:, kt, :],
                                        start=(kt == 0),
                                        stop=(kt == KH - 1),
                                    )
                            # DVE may read only ONE PSUM operand per op: stage
                            # P1 to SBUF on the (idle) scalar engine first.
                            for hh in range(HPC):
                                sl = slice(hh * 64, hh * 64 + 64)
                                P1 = ps3[sl, 0, :]
                                P2 = ps3[sl, 1, :]
                                P3 = ps3[sl, 2, :]
                                dre = dst[b, hh][0:64, csl]
                                dim = dst[b, hh][64:128, csl]
                                s1 = stg.tile([64, TF], F32, name="s1", tag="s1")
                                nc.scalar.activation(
                                    s1[:], P1, mybir.ActivationFunctionType.Identity
                                )
                                tmp = stg.tile([64, TF], F32, name="tmp", tag="tmp")
                                if is_q:
                                    nc.vector.scalar_tensor_tensor(
                                        dre, s1[:], bq_sb[:, 2 * hh : 2 * hh + 1], P2, ADD, SUB
                                    )
                                    # im = (P3 + b_im - P1) - P2
                                    nc.vector.scalar_tensor_tensor(
                                        tmp[:], P3, bq_sb[:, 2 * hh + 1 : 2 * hh + 2], s1[:], ADD, SUB
                                    )
                                    nc.vector.tensor_tensor(dim, tmp[:], P2, SUB)
                                else:
                                    # k bias cancels in softmax — pure combines
                                    nc.vector.tensor_tensor(dre, s1[:], P2, SUB)
                                    nc.vector.tensor_tensor(tmp[:], P3, s1[:], SUB)
                                    nc.vector.tensor_tensor(dim, tmp[:], P2, SUB)
                        # v: stacked form, [tokens, channels] layout
                        for m in range(TF // 128):
                            vp = v_ps.tile(
                                [128, 2 * HPC * D], F32, name="vps", tag="vps"
                            )
                            for kt in range(KT):
                                nc.tensor.matmul(
                                    vp[:],
                                    xt[:, kt, m * 128 : (m + 1) * 128],
                                    wv_sb[:, kt, :],
                                    start=(kt == 0),
                                    stop=(kt == KT - 1),
                                )
                            ktok = ci * (TF // 128) + m
                            for hh in range(HPC):
                                nc.vector.tensor_copy(
                                    vt[b, hh][:, ktok, :],
                                    vp[:, hh * 128 : (hh + 1) * 128],
                                )
                    ctxP.close()

                    # ---- o-projection loads (overlap attention) ----
                    ctxM = ExitStack()
                    mp = ctxM.enter_context(tc.tile_pool(name="mp", bufs=1))
                    m3_sb = mp.tile([128, 3, KH, C], BF16, name="m3sb", tag="m3sb")
                    nc.gpsimd.dma_start(m3_sb[:], m3_d[:])
                    oc = ctxM.enter_context(tc.tile_pool(name="oc", bufs=1))
                    at_sb, atsum = {}, {}
                    for b in range(B):
                        at_sb[b] = oc.tile(
                            [128, KH, 2, TOKB], BF16, name=f"at{b}", tag=f"at{b}"
                        )
                        atsum[b] = oc.tile(
                            [128, KH, TOKB], BF16, name=f"ats{b}", tag=f"ats{b}"
                        )

                    # ---- attention, software-pipelined one kt-pair ahead ----
                    ctxA = ExitStack()
                    expp = ctxA.enter_context(tc.tile_pool(name="expp", bufs=3))
                    evp = ctxA.enter_context(tc.tile_pool(name="evp", bufs=3))
                    sc_ps = ctxA.enter_context(
                        tc.tile_pool(name="sc_ps", bufs=2, space="PSUM")
                    )
                    av_ps = ctxA.enter_context(
                        tc.tile_pool(name="av_ps", bufs=2, space="PSUM")
                    )
                    den_ps = ctxA.enter_context(
                        tc.tile_pool(name="den_ps", bufs=2, space="PSUM")
                    )
                    NKP = N // 256  # key-tile pairs
                    units = [
                        (b, hh, qt)
                        for b in range(B)
                        for hh in range(HPC)
                        for qt in range(N // 512)
                    ]
                    jobs = [(ui, ktp) for ui in range(len(units)) for ktp in range(NKP)]

                    def emit_sc(job):
                        ui, ktp = job
                        b, hh, qt = units[ui]
                        qsl = slice(qt * 512, qt * 512 + 512)
                        sc = sc_ps.tile([128, 1024], F32, name="scps", tag="scps")
                        for half in range(2):
                            kt = ktp * 2 + half
                            nc.tensor.matmul(
                                sc[:, half * 512 : half * 512 + 512],
                                kc[b, hh][:, kt * 128 : (kt + 1) * 128],
                                qc[b, hh][:, qsl],
                                start=True,
                                stop=True,
                            )
                        ex = expp.tile([128, 1024], BF16, name="ex", tag="ex")
                        nc.scalar.activation(
                            ex[:], sc[:], mybir.ActivationFunctionType.Exp, scale=0.125
                        )
                        # pair-sum on DVE so den contracts once per pair
                        exs = expp.tile([128, 512], BF16, name="exs", tag="exs")
                        nc.vector.tensor_tensor(
                            exs[:], ex[:, 0:512], ex[:, 512:1024], ADD
                        )
                        return ex, exs

                    pend = {0: emit_sc(jobs[0])}
                    acc = {}
                    for j, (ui, ktp) in enumerate(jobs):
                        if j + 1 < len(jobs):
                            pend[j + 1] = emit_sc(jobs[j + 1])
                        b, hh, qt = units[ui]
                        ex, exs = pend.pop(j)
                        if ktp == 0:
                            acc[ui] = (
                                av_ps.tile([128, 512], F32, name="avps", tag="avps"),
                                den_ps.tile([128, 512], F32, name="denps", tag="denps"),
                            )
                        av, den = acc[ui]
                        for half in range(2):
                            kt = ktp * 2 + half
                            first = ktp == 0 and half == 0
                            last = ktp == NKP - 1 and half == 1
                            exh = ex[:, half * 512 : half * 512 + 512]
                            nc.tensor.matmul(
                                av[:], vt[b, hh][:, kt, :], exh, start=first, stop=last
                            )
                        nc.tensor.matmul(
                            den[:],
                            ones128[:],
                            exs[:],
                            start=(ktp == 0),
                            stop=(ktp == NKP - 1),
                        )
                        if ktp == NKP - 1:
                            del acc[ui]
                            rb = evp.tile([128, 512], F32, name="rb", tag="rb")
                            # den is a sum of exps in [~1, ~1e5]: no edge cases,
                            # and ~18 correct bits beats the bf16 noise. 5x
                            # faster than reciprocal() so it doesn't
                            # head-of-line-block the DVE queue.
                            nc.vector.reciprocal_approx_fast(rb[:], den[:])
                            outc = evp.tile([128, 512], BF16, name="outc", tag="outc")
                            nc.vector.tensor_tensor(
                                outc[:], av[:], rb[:], mybir.AluOpType.mult
                            )
                            # emit re rows and im rows to separate channel
                            # blocks so the o-projection contracts full-width
                            # re/im tiles (3M form)
                            for s in range(2):
                                tsl = slice(s * 256, s * 256 + 256)
                                nc.sync.dma_start(
                                    out_d[b][qt * 2 + s, hh * 64 : (hh + 1) * 64, :],
                                    outc[0:64, tsl],
                                )
                                nc.sync.dma_start(
                                    out_d[b][
                                        qt * 2 + s, 128 + hh * 64 : 128 + (hh + 1) * 64, :
                                    ],
                                    outc[64:128, tsl],
                                )
                            if (ui + 1) % (len(units) // B) == 0:
                                nc.gpsimd.collective_compute(
                                    "AllToAll",
                                    mybir.AluOpType.bypass,
                                    replica_groups=[list(range(NCORES))],
                                    ins=[out_d[b].opt()],
                                    outs=[at_d[b].opt()],
                                )
                                nc.gpsimd.dma_start(
                                    at_sb[b][:],
                                    at_d[b].rearrange("r (g p) t -> p r g t", p=128),
                                )
                    ctxA.close()

                    # ---- output projection (3M): 256-token slice per batch ----
                    with (
                        tc.tile_pool(name="oev", bufs=3) as oev,
                        tc.tile_pool(name="o3_ps", bufs=2, space="PSUM") as o3_ps,
                    ):
                        for b in range(B):
                            nc.vector.tensor_tensor(
                                atsum[b][:], at_sb[b][:, :, 0, :], at_sb[b][:, :, 1, :], ADD
                            )
                            for ot in range(C // 128):
                                osl = slice(ot * 128, (ot + 1) * 128)
                                ps3 = o3_ps.tile([128, 3, TOKB], F32, name="o3", tag="o3")
                                for s in range(3):
                                    for r in range(KH):
                                        src = (
                                            atsum[b][:, r, :]
                                            if s == 2
                                            else at_sb[b][:, r, s, :]
                                        )
                                        nc.tensor.matmul(
                                            ps3[:, s, :],
                                            m3_sb[:, s, r, osl],
                                            src,
                                            start=(r == 0),
                                            stop=(r == KH - 1),
                                        )
                                # stage P1 via scalar (DVE: one PSUM input max)
                                so1 = oev.tile([128, TOKB], F32, name="so1", tag="so1")
                                nc.scalar.activation(
                                    so1[:],
                                    ps3[:, 0, :],
                                    mybir.ActivationFunctionType.Identity,
                                )
                                y_re = oev.tile([128, TOKB], F32, name="yre", tag="yre")
                                nc.vector.scalar_tensor_tensor(
                                    y_re[:],
                                    so1[:],
                                    bore_sb[:, ot : ot + 1],
                                    ps3[:, 1, :],
                                    ADD,
                                    SUB,
                                )
                                nc.sync.dma_start(
                                    yout_d[osl, b * TOKB : (b + 1) * TOKB], y_re[:]
                                )
                                tmp = oev.tile([128, TOKB], F32, name="otmp", tag="otmp")
                                nc.vector.scalar_tensor_tensor(
                                    tmp[:],
                                    ps3[:, 2, :],
                                    boim_sb[:, ot : ot + 1],
                                    so1[:],
                                    ADD,
                                    SUB,
                                )
                                y_im = oev.tile([128, TOKB], F32, name="yim", tag="yim")
                                nc.vector.tensor_tensor(
                                    y_im[:], tmp[:], ps3[:, 1, :], SUB
                                )
                                nc.sync.dma_start(
                                    yout_d[C + ot * 128 : C + (ot + 1) * 128,
                                           b * TOKB : (b + 1) * TOKB],
                                    y_im[:],
                                )
                    ctxM.close()
    nc.compile()
    return nc


_NC_CACHE = None


def _get_program():
    global _NC_CACHE
    if _NC_CACHE is None:
        _NC_CACHE = _build_program()
    return _NC_CACHE


def _run(inputs, trace=False, trace_kwargs=None):
    shared, per_core = _host_prep(inputs)
    nc = _get_program()
    in_maps = []
    for c in range(NCORES):
        d = per_core[c]
        in_maps.append(
            {
                "xc": shared["xc"],
                "wq": d["wq"],
                "wk": d["wk"],
                "wv": d["wv"],
                "bq": d["bq"],
                "m3": shared["m3"],
                "bo_re": shared["bo_re"],
                "bo_im": shared["bo_im"],
            }
        )
    res = run_bass_kernel_spmd(
        nc, in_maps, list(range(NCORES)), trace=trace, **(trace_kwargs or {})
    )
    youts = [res.results[c]["yout"] for c in range(NCORES)]
    re = np.empty((C, B, N), dtype=np.float32)
    im = np.empty((C, B, N), dtype=np.float32)
    for c in range(NCORES):
        y = youts[c]
        tsl = slice(c * TOKB, (c + 1) * TOKB)
        re[:, 0, tsl] = y[:C, :TOKB]
        re[:, 1, tsl] = y[:C, TOKB:]
        im[:, 0, tsl] = y[C:, :TOKB]
        im[:, 1, tsl] = y[C:, TOKB:]
    out = np.stack([re.transpose(1, 2, 0), im.transpose(1, 2, 0)]).astype(np.float32)
    return out, res


def kernel(**inputs) -> np.ndarray:
    out, _ = _run(inputs, trace=False)
    return out
